# revision 1
# baseline (speedup 1.0000x reference)
"""Trainium2 Bass kernel: 3x3 valid 2D cross-correlation on an 8192x8192 f32 image.

Strategy (8 NeuronCores, pure spatial/data parallel):
  - Row-shard on the host: core i receives input rows [1024*i, 1024*i + 1026)
    (the 2-row halo is free since we shard from the full input; rows past the
    bottom edge are zero-padded and the corresponding outputs discarded).
  - Per core, raw-bass pipeline (manual semaphores, no framework preamble):
    9 row-tiles (128 input partitions -> 126 output rows; last tile 18->16).
    For each tile, 16 column chunks of 512; per chunk 3 TensorEngine matmuls
    accumulate into a PSUM bank:
        out[y, c] = sum_dx (M_dx.T @ X)[y, c+dx]
    where M_dx[k, y] = w[k-y, dx] is a 3-diagonal band matrix built on the
    host from the 3x3 weight. Matmul operands use float32r (fp32 bits on the
    fast 4-byte PE streaming path, ~1 cycle/column, ~1e-4 rel err).
  - ScalarE copies even chunks PSUM->SBUF, VectorE odd chunks; SP ring does
    x loads, ACT ring does y stores (full-width rows; the last two tiles
    store in halves to drain the tail at fine granularity).
  - The kernel is HBM-bandwidth-bound: DMA measured busy end-to-end at
    ~360 GB/s per core with no idle gaps.
"""

import numpy as np

import concourse.bass as bass
import concourse.mybir as mybir
from concourse.bass_utils import run_bass_kernel_spmd

H = W = 8192
KH = KW = 3
N_CORES = 8
OUT_H = H - KH + 1  # 8190
OUT_W = W - KW + 1  # 8190

ROWS_PER_CORE = 1024          # output rows per core (core 7: keep 1022)
IN_ROWS_PER_CORE = ROWS_PER_CORE + KH - 1  # 1026
TILE_OUT = 126                # output rows per 128-partition input tile
CHUNK = 512                   # PSUM bank width (fp32)
N_TILES = 9
N_CHUNKS = 16
HALF_W = 4096
XBUFS = 3
OBUFS = 2

_NC_CACHE = {}


def _build_program():
    nc = bass.Bass("TRN2", target_bir_lowering=False, debug=False)
    x = nc.declare_dram_parameter(
        "x", [IN_ROWS_PER_CORE, W], mybir.dt.float32r, isOutput=False
    )
    m = nc.declare_dram_parameter(
        "m", [128, 3 * TILE_OUT], mybir.dt.float32r, isOutput=False
    )
    y = nc.declare_dram_parameter(
        "y", [ROWS_PER_CORE, OUT_W], mybir.dt.float32, isOutput=True
    )

    xb = [nc.alloc_sbuf_tensor(f"xb{i}", [128, W], mybir.dt.float32r).ap()
          for i in range(XBUFS)]
    ob = [nc.alloc_sbuf_tensor(f"ob{i}", [128, OUT_W], mybir.dt.float32).ap()
          for i in range(OBUFS)]
    mt = nc.alloc_sbuf_tensor("mt", [128, 3 * TILE_OUT], mybir.dt.float32r).ap()
    pb = [nc.alloc_psum_tensor(f"pb{i}", [128, CHUNK], mybir.dt.float32).ap()
          for i in range(8)]

    sx = [nc.alloc_semaphore(f"sx{t}") for t in range(N_TILES)]
    sm = nc.alloc_semaphore("sm")
    s_mm = nc.alloc_semaphore("s_mm")
    s_cpA = nc.alloc_semaphore("s_cpA")
    s_cpD = nc.alloc_semaphore("s_cpD")
    sst = [nc.alloc_semaphore(f"sst{j}") for j in range(N_TILES)]

    def rows_of(t):
        rows_out = min(TILE_OUT, ROWS_PER_CORE - t * TILE_OUT)
        return rows_out, rows_out + KH - 1

    with nc.Block() as block:

        @block.sync
        def _(sync):
            for t in range(N_TILES):
                r0 = t * TILE_OUT
                _, rows_in = rows_of(t)
                if t == 1:
                    sync.dma_start(out=mt, in_=m[:]).then_inc(sm, 16)
                if t >= XBUFS:
                    # x slot reuse: previous tile in this slot fully consumed
                    sync.wait_ge(s_mm, 16 * (t - XBUFS + 1))
                sync.dma_start(
                    out=xb[t % XBUFS][:rows_in], in_=x[r0:r0 + rows_in, :]
                ).then_inc(sx[t], 16)
            for t in range(N_TILES):
                sync.wait_ge(sst[t], 32 if t >= N_TILES - 2 else 16)

        @block.tensor
        def _(tensor):
            tensor.wait_ge(sm, 16)
            for t in range(N_TILES):
                rows_out, rows_in = rows_of(t)
                tensor.wait_ge(sx[t], 16)
                for k in range(N_CHUNKS):
                    g = t * N_CHUNKS + k
                    b = g % 8
                    if g >= 8:
                        # PSUM bank b free once chunk g-8's copy retired
                        tp, kp = divmod(g - 8, N_CHUNKS)
                        if kp % 2 == 0:
                            tensor.wait_ge(s_cpA, 8 * tp + kp // 2 + 1)
                        else:
                            tensor.wait_ge(s_cpD, 8 * tp + (kp - 1) // 2 + 1)
                    c0 = k * CHUNK
                    wid = min(CHUNK, OUT_W - c0)
                    for dx in range(KW):
                        ins = nc.tensor.matmul(
                            pb[b][:rows_out, :wid],
                            mt[:rows_in, dx * TILE_OUT:dx * TILE_OUT + rows_out],
                            xb[t % XBUFS][:rows_in, c0 + dx:c0 + dx + wid],
                            start=(dx == 0),
                            stop=(dx == KW - 1),
                        )
                        if dx == KW - 1:
                            ins.then_inc(s_mm, 1)

        @block.scalar
        def _(scalar):
            for t in range(N_TILES):
                rows_out, _ = rows_of(t)
                r0 = t * TILE_OUT
                if t >= OBUFS:
                    scalar.wait_ge(sst[t - OBUFS], 16)

                def act_copy(k):
                    g = t * N_CHUNKS + k
                    c0 = k * CHUNK
                    wid = min(CHUNK, OUT_W - c0)
                    scalar.wait_ge(s_mm, g + 1)
                    nc.scalar.copy(
                        out=ob[t % OBUFS][:rows_out, c0:c0 + wid],
                        in_=pb[g % 8][:rows_out, :wid],
                    ).then_inc(s_cpA, 1)

                if t < N_TILES - 2:
                    for k in range(0, N_CHUNKS, 2):
                        act_copy(k)
                    scalar.wait_ge(s_cpA, 8 * (t + 1))
                    scalar.wait_ge(s_cpD, 8 * (t + 1))
                    scalar.dma_start(
                        out=y[r0:r0 + rows_out, :],
                        in_=ob[t % OBUFS][:rows_out, :],
                    ).then_inc(sst[t], 16)
                else:
                    # drain tiles: store halves as soon as each is copied
                    for k in range(0, N_CHUNKS // 2, 2):
                        act_copy(k)
                    scalar.wait_ge(s_cpA, 8 * t + 4)
                    scalar.wait_ge(s_cpD, 8 * t + 4)
                    scalar.dma_start(
                        out=y[r0:r0 + rows_out, :HALF_W],
                        in_=ob[t % OBUFS][:rows_out, :HALF_W],
                    ).then_inc(sst[t], 16)
                    for k in range(N_CHUNKS // 2, N_CHUNKS, 2):
                        act_copy(k)
                    scalar.wait_ge(s_cpA, 8 * (t + 1))
                    scalar.wait_ge(s_cpD, 8 * (t + 1))
                    scalar.dma_start(
                        out=y[r0:r0 + rows_out, HALF_W:],
                        in_=ob[t % OBUFS][:rows_out, HALF_W:OUT_W],
                    ).then_inc(sst[t], 16)

        @block.vector
        def _(vector):
            for t in range(N_TILES):
                rows_out, _ = rows_of(t)
                if t >= OBUFS:
                    vector.wait_ge(sst[t - OBUFS], 16)
                for k in range(1, N_CHUNKS, 2):
                    g = t * N_CHUNKS + k
                    c0 = k * CHUNK
                    wid = min(CHUNK, OUT_W - c0)
                    vector.wait_ge(s_mm, g + 1)
                    nc.vector.tensor_copy(
                        out=ob[t % OBUFS][:rows_out, c0:c0 + wid],
                        in_=pb[g % 8][:rows_out, :wid],
                    ).then_inc(s_cpD, 1)

    return nc


def _get_program():
    if "nc" not in _NC_CACHE:
        _NC_CACHE["nc"] = _build_program()
    return _NC_CACHE["nc"]


def _band_matrices(weight: np.ndarray) -> np.ndarray:
    """m[k, dx*126 + y] = w[k-y, dx] for 0 <= k-y < 3."""
    mm = np.zeros((128, 3 * TILE_OUT), dtype=np.float32)
    for dx in range(KW):
        for dy in range(KH):
            ys = np.arange(TILE_OUT)
            mm[ys + dy, dx * TILE_OUT + ys] = weight[dy, dx]
    return mm


def _in_maps(x, weight):
    mmat = _band_matrices(weight)
    maps = []
    for i in range(N_CORES):
        r0 = i * ROWS_PER_CORE
        r1 = min(r0 + IN_ROWS_PER_CORE, H)
        shard = np.zeros((IN_ROWS_PER_CORE, W), dtype=np.float32)
        shard[: r1 - r0] = x[r0:r1]
        maps.append({"x": shard, "m": mmat})
    return maps


def kernel(x: np.ndarray, weight: np.ndarray) -> np.ndarray:
    x = np.ascontiguousarray(np.asarray(x, dtype=np.float32))
    weight = np.asarray(weight, dtype=np.float32)
    assert x.shape == (H, W) and weight.shape == (KH, KW)

    nc = _get_program()
    res = run_bass_kernel_spmd(nc, _in_maps(x, weight),
                               core_ids=list(range(N_CORES)))

    out = np.empty((OUT_H, OUT_W), dtype=np.float32)
    for i in range(N_CORES):
        r0 = i * ROWS_PER_CORE
        keep = min(ROWS_PER_CORE, OUT_H - r0)
        out[r0:r0 + keep] = res.results[i]["y"][:keep]
    return out



# revision 2
# speedup vs baseline: 1.4953x; 1.4953x over previous
"""Trainium2 Bass kernel: 3x3 valid 2D cross-correlation on an 8192x8192 f32 image.

Strategy (8 NeuronCores, pure spatial/data parallel):
  - Row-shard on the host: core i receives input rows [1024*i, 1024*i + 1026)
    (the 2-row halo is free since we shard from the full input; rows past the
    bottom edge are zero-padded and the corresponding outputs discarded).
  - fp16 I/O: the harness tolerance is 2e-2; casting x/w/y to fp16 halves HBM
    traffic (the kernel is HBM-bandwidth-bound) at ~2e-4 rel err. The host
    casts x to fp16, the device returns fp16 y, the host upcasts to fp32.
  - Per core, raw-bass pipeline (manual semaphores, no framework preamble):
    9 row-tiles (128 input partitions -> 126 output rows; last tile 18->16).
    For each tile, 16 column chunks of 512; per chunk 3 TensorEngine matmuls
    accumulate into a PSUM bank:
        out[y, c] = sum_dx (M_dx.T @ X)[y, c+dx]
    where M_dx[k, y] = w[k-y, dx] is a 3-diagonal band matrix built on the
    host from the 3x3 weight.
  - ScalarE copies even chunks PSUM->SBUF (casting fp32->fp16), VectorE odd
    chunks; SP ring does x loads, ACT ring does y stores (full-width rows;
    the last two tiles store in halves to drain the tail at fine granularity).
"""

import numpy as np

import concourse.bass as bass
import concourse.mybir as mybir
from concourse.bass_utils import run_bass_kernel_spmd

H = W = 8192
KH = KW = 3
N_CORES = 8
OUT_H = H - KH + 1  # 8190
OUT_W = W - KW + 1  # 8190

ROWS_PER_CORE = 1024          # output rows per core (core 7: keep 1022)
IN_ROWS_PER_CORE = ROWS_PER_CORE + KH - 1  # 1026
TILE_OUT = 126                # output rows per 128-partition input tile
CHUNK = 512                   # PSUM bank width (fp32)
N_TILES = 9
N_CHUNKS = 16
HALF_W = 4096
XBUFS = 3
OBUFS = 2

_NC_CACHE = {}


def _build_program():
    nc = bass.Bass("TRN2", target_bir_lowering=False, debug=False)
    x = nc.declare_dram_parameter(
        "x", [IN_ROWS_PER_CORE, W], mybir.dt.float16, isOutput=False
    )
    m = nc.declare_dram_parameter(
        "m", [128, 3 * TILE_OUT], mybir.dt.float16, isOutput=False
    )
    y = nc.declare_dram_parameter(
        "y", [ROWS_PER_CORE, OUT_W], mybir.dt.float16, isOutput=True
    )

    xb = [nc.alloc_sbuf_tensor(f"xb{i}", [128, W], mybir.dt.float16).ap()
          for i in range(XBUFS)]
    ob = [nc.alloc_sbuf_tensor(f"ob{i}", [128, OUT_W], mybir.dt.float16).ap()
          for i in range(OBUFS)]
    mt = nc.alloc_sbuf_tensor("mt", [128, 3 * TILE_OUT], mybir.dt.float16).ap()
    pb = [nc.alloc_psum_tensor(f"pb{i}", [128, CHUNK], mybir.dt.float32).ap()
          for i in range(8)]

    sx = [nc.alloc_semaphore(f"sx{t}") for t in range(N_TILES)]
    sm = nc.alloc_semaphore("sm")
    s_mm = nc.alloc_semaphore("s_mm")
    s_cpA = nc.alloc_semaphore("s_cpA")
    s_cpD = nc.alloc_semaphore("s_cpD")
    sst = [nc.alloc_semaphore(f"sst{j}") for j in range(N_TILES)]

    def rows_of(t):
        rows_out = min(TILE_OUT, ROWS_PER_CORE - t * TILE_OUT)
        return rows_out, rows_out + KH - 1

    with nc.Block() as block:

        @block.sync
        def _(sync):
            for t in range(N_TILES):
                r0 = t * TILE_OUT
                _, rows_in = rows_of(t)
                if t == 1:
                    sync.dma_start(out=mt, in_=m[:]).then_inc(sm, 16)
                if t >= XBUFS:
                    # x slot reuse: previous tile in this slot fully consumed
                    sync.wait_ge(s_mm, 16 * (t - XBUFS + 1))
                sync.dma_start(
                    out=xb[t % XBUFS][:rows_in], in_=x[r0:r0 + rows_in, :]
                ).then_inc(sx[t], 16)
            for t in range(N_TILES):
                sync.wait_ge(sst[t], 32 if t >= N_TILES - 2 else 16)

        @block.tensor
        def _(tensor):
            tensor.wait_ge(sm, 16)
            for t in range(N_TILES):
                rows_out, rows_in = rows_of(t)
                tensor.wait_ge(sx[t], 16)
                for k in range(N_CHUNKS):
                    g = t * N_CHUNKS + k
                    b = g % 8
                    if g >= 8:
                        # PSUM bank b free once chunk g-8's copy retired
                        tp, kp = divmod(g - 8, N_CHUNKS)
                        if kp % 2 == 0:
                            tensor.wait_ge(s_cpA, 8 * tp + kp // 2 + 1)
                        else:
                            tensor.wait_ge(s_cpD, 8 * tp + (kp - 1) // 2 + 1)
                    c0 = k * CHUNK
                    wid = min(CHUNK, OUT_W - c0)
                    for dx in range(KW):
                        ins = nc.tensor.matmul(
                            pb[b][:rows_out, :wid],
                            mt[:rows_in, dx * TILE_OUT:dx * TILE_OUT + rows_out],
                            xb[t % XBUFS][:rows_in, c0 + dx:c0 + dx + wid],
                            start=(dx == 0),
                            stop=(dx == KW - 1),
                        )
                        if dx == KW - 1:
                            ins.then_inc(s_mm, 1)

        @block.scalar
        def _(scalar):
            for t in range(N_TILES):
                rows_out, _ = rows_of(t)
                r0 = t * TILE_OUT
                if t >= OBUFS:
                    scalar.wait_ge(sst[t - OBUFS], 16)

                def act_copy(k):
                    g = t * N_CHUNKS + k
                    c0 = k * CHUNK
                    wid = min(CHUNK, OUT_W - c0)
                    scalar.wait_ge(s_mm, g + 1)
                    nc.scalar.copy(
                        out=ob[t % OBUFS][:rows_out, c0:c0 + wid],
                        in_=pb[g % 8][:rows_out, :wid],
                    ).then_inc(s_cpA, 1)

                if t < N_TILES - 2:
                    for k in range(0, N_CHUNKS, 2):
                        act_copy(k)
                    scalar.wait_ge(s_cpA, 8 * (t + 1))
                    scalar.wait_ge(s_cpD, 8 * (t + 1))
                    scalar.dma_start(
                        out=y[r0:r0 + rows_out, :],
                        in_=ob[t % OBUFS][:rows_out, :],
                    ).then_inc(sst[t], 16)
                else:
                    # drain tiles: store halves as soon as each is copied
                    for k in range(0, N_CHUNKS // 2, 2):
                        act_copy(k)
                    scalar.wait_ge(s_cpA, 8 * t + 4)
                    scalar.wait_ge(s_cpD, 8 * t + 4)
                    scalar.dma_start(
                        out=y[r0:r0 + rows_out, :HALF_W],
                        in_=ob[t % OBUFS][:rows_out, :HALF_W],
                    ).then_inc(sst[t], 16)
                    for k in range(N_CHUNKS // 2, N_CHUNKS, 2):
                        act_copy(k)
                    scalar.wait_ge(s_cpA, 8 * (t + 1))
                    scalar.wait_ge(s_cpD, 8 * (t + 1))
                    scalar.dma_start(
                        out=y[r0:r0 + rows_out, HALF_W:],
                        in_=ob[t % OBUFS][:rows_out, HALF_W:OUT_W],
                    ).then_inc(sst[t], 16)

        @block.vector
        def _(vector):
            for t in range(N_TILES):
                rows_out, _ = rows_of(t)
                if t >= OBUFS:
                    vector.wait_ge(sst[t - OBUFS], 16)
                for k in range(1, N_CHUNKS, 2):
                    g = t * N_CHUNKS + k
                    c0 = k * CHUNK
                    wid = min(CHUNK, OUT_W - c0)
                    vector.wait_ge(s_mm, g + 1)
                    nc.vector.tensor_copy(
                        out=ob[t % OBUFS][:rows_out, c0:c0 + wid],
                        in_=pb[g % 8][:rows_out, :wid],
                    ).then_inc(s_cpD, 1)

    return nc


def _get_program():
    if "nc" not in _NC_CACHE:
        _NC_CACHE["nc"] = _build_program()
    return _NC_CACHE["nc"]


def _band_matrices(weight: np.ndarray) -> np.ndarray:
    """m[k, dx*126 + y] = w[k-y, dx] for 0 <= k-y < 3."""
    mm = np.zeros((128, 3 * TILE_OUT), dtype=np.float16)
    for dx in range(KW):
        for dy in range(KH):
            ys = np.arange(TILE_OUT)
            mm[ys + dy, dx * TILE_OUT + ys] = weight[dy, dx]
    return mm


def _in_maps(x, weight):
    mmat = _band_matrices(weight)
    x16 = x.astype(np.float16)
    maps = []
    for i in range(N_CORES):
        r0 = i * ROWS_PER_CORE
        r1 = min(r0 + IN_ROWS_PER_CORE, H)
        shard = np.zeros((IN_ROWS_PER_CORE, W), dtype=np.float16)
        shard[: r1 - r0] = x16[r0:r1]
        maps.append({"x": shard, "m": mmat})
    return maps


def kernel(x: np.ndarray, weight: np.ndarray) -> np.ndarray:
    x = np.ascontiguousarray(np.asarray(x, dtype=np.float32))
    weight = np.asarray(weight, dtype=np.float32)
    assert x.shape == (H, W) and weight.shape == (KH, KW)

    nc = _get_program()
    res = run_bass_kernel_spmd(nc, _in_maps(x, weight),
                               core_ids=list(range(N_CORES)))

    out = np.empty((OUT_H, OUT_W), dtype=np.float32)
    for i in range(N_CORES):
        r0 = i * ROWS_PER_CORE
        keep = min(ROWS_PER_CORE, OUT_H - r0)
        out[r0:r0 + keep] = res.results[i]["y"][:keep].astype(np.float32)
    return out


# revision 4
# speedup vs baseline: 1.6881x; 1.1289x over previous
"""Trainium2 Bass kernel: 3x3 valid 2D cross-correlation on an 8192x8192 f32 image.

Strategy (8 NeuronCores, pure spatial/data parallel):
  - Row-shard on the host: core i receives input rows [1024*i, 1024*i + 1026)
    (the 2-row halo is free since we shard from the full input; rows past the
    bottom edge are zero-padded and the corresponding outputs discarded).
  - fp16 I/O: the harness tolerance is 2e-2; casting x/w/y to fp16 halves HBM
    traffic (the kernel is HBM-bandwidth-bound) at ~3e-4 rel err. The host
    casts x to fp16, the device returns fp16 y, the host upcasts to fp32.
  - Per core: 8 full row-tiles (128 input partitions -> 126 output rows).
    For each tile, 16 column chunks of 512; per chunk 3 TensorEngine matmuls
    accumulate into a PSUM bank:
        out[y, c] = sum_dx (M_dx.T @ X)[y, c+dx]
    where M_dx[k, y] = w[k-y, dx] is a 3-diagonal band matrix built on the
    host from the 3x3 weight.
  - Stub tile (last 16 rows): the 3 dx-shifted 18-row input slices are DMA'd
    into partitions 0..53 (shift folded into the load), so each chunk is a
    single K=54 matmul -- 16 matmuls instead of 48.
  - ScalarE copies even chunks PSUM->SBUF (casting fp32->fp16), VectorE odd
    chunks; SP ring does x loads, ACT ring does the m load + y stores (every
    tile stores in half-width pieces as soon as that half is copied).
  - Tile 0's load is split into 4 column pieces (piece 0 on the ACT ring) so
    the first matmul can start ~5us earlier.
"""

import numpy as np

import concourse.bass as bass
import concourse.mybir as mybir
from concourse.bass_utils import run_bass_kernel_spmd

H = W = 8192
KH = KW = 3
N_CORES = 8
OUT_H = H - KH + 1  # 8190
OUT_W = W - KW + 1  # 8190

ROWS_PER_CORE = 1024          # output rows per core (core 7: keep 1022)
IN_ROWS_PER_CORE = ROWS_PER_CORE + KH - 1  # 1026
TILE_OUT = 126                # output rows per 128-partition input tile
CHUNK = 512                   # PSUM bank width (fp32)
N_FULL = 8                    # full tiles; stub tile 8 covers rows 1008..1023
N_TILES = 9
STUB_R0 = N_FULL * TILE_OUT   # 1008
STUB_OUT = ROWS_PER_CORE - STUB_R0  # 16
STUB_IN = STUB_OUT + KH - 1   # 18
N_CHUNKS = 16
HALF_W = 4096
PIECE = 2048                  # tile-0 load piece width (4 pieces)
XBUFS = 4
OBUFS = 2
MCOLS = 3 * TILE_OUT + STUB_OUT  # 394

_NC_CACHE = {}


def _build_program():
    nc = bass.Bass("TRN2", target_bir_lowering=False, debug=False)
    x = nc.declare_dram_parameter(
        "x", [IN_ROWS_PER_CORE, W], mybir.dt.float16, isOutput=False
    )
    m = nc.declare_dram_parameter(
        "m", [128, MCOLS], mybir.dt.float16, isOutput=False
    )
    y = nc.declare_dram_parameter(
        "y", [ROWS_PER_CORE, OUT_W], mybir.dt.float16, isOutput=True
    )

    xb = [nc.alloc_sbuf_tensor(f"xb{i}", [128, W], mybir.dt.float16).ap()
          for i in range(XBUFS)]
    ob = [nc.alloc_sbuf_tensor(f"ob{i}", [128, OUT_W], mybir.dt.float16).ap()
          for i in range(OBUFS)]
    mt = nc.alloc_sbuf_tensor("mt", [128, MCOLS], mybir.dt.float16).ap()
    pb = [nc.alloc_psum_tensor(f"pb{i}", [128, CHUNK], mybir.dt.float32).ap()
          for i in range(8)]

    sx = [nc.alloc_semaphore(f"sx{t}") for t in range(N_TILES)]
    s_p0 = nc.alloc_semaphore("s_p0")   # tile-0 piece 0 (ACT ring)
    sm = nc.alloc_semaphore("sm")
    s_mm = nc.alloc_semaphore("s_mm")
    s_cpA = nc.alloc_semaphore("s_cpA")
    s_cpD = nc.alloc_semaphore("s_cpD")
    sst = [nc.alloc_semaphore(f"sst{j}") for j in range(N_TILES)]

    def need_piece(k):
        # highest sync-ring piece (1..3) whose columns chunk k of tile 0 needs
        return min(3, (k * CHUNK + CHUNK + KW - 1) // PIECE)

    with nc.Block() as block:

        @block.sync
        def _(sync):
            # tile 0: pieces 1..3 (piece 0 goes on the ACT ring)
            for p in range(1, 4):
                sync.dma_start(
                    out=xb[0][:, p * PIECE:(p + 1) * PIECE],
                    in_=x[0:128, p * PIECE:(p + 1) * PIECE],
                ).then_inc(sx[0], 16)
            for t in range(1, N_FULL):
                r0 = t * TILE_OUT
                if t >= XBUFS:
                    # x slot reuse: previous tile in this slot fully consumed
                    sync.wait_ge(s_mm, 16 * (t - XBUFS + 1))
                sync.dma_start(
                    out=xb[t % XBUFS][:128], in_=x[r0:r0 + 128, :]
                ).then_inc(sx[t], 16)
            # stub tile: 3 dx-shifted replicas of the last 18 rows into
            # partitions {0..17, 18..35, 36..53} of slot 0
            sync.wait_ge(s_mm, 16 * (N_FULL - XBUFS + 1))
            for dx in range(KW):
                sync.dma_start(
                    out=xb[N_FULL % XBUFS][dx * STUB_IN:(dx + 1) * STUB_IN,
                                           0:W - dx],
                    in_=x[STUB_R0:STUB_R0 + STUB_IN, dx:W],
                ).then_inc(sx[N_FULL], 16)
            for t in range(N_TILES):
                sync.wait_ge(sst[t], 32)

        @block.tensor
        def _(tensor):
            tensor.wait_ge(sm, 16)
            tensor.wait_ge(s_p0, 16)
            for t in range(N_TILES):
                stub = t == N_FULL
                rows_out = STUB_OUT if stub else TILE_OUT
                if stub:
                    tensor.wait_ge(sx[t], 48)
                elif t > 0:
                    tensor.wait_ge(sx[t], 16)
                for k in range(N_CHUNKS):
                    g = t * N_CHUNKS + k
                    b = g % 8
                    if t == 0 and k > 0 and need_piece(k) > need_piece(k - 1):
                        tensor.wait_ge(sx[0], 16 * need_piece(k))
                    if g >= 8:
                        # PSUM bank b free once chunk g-8's copy retired
                        tp, kp = divmod(g - 8, N_CHUNKS)
                        if kp % 2 == 0:
                            tensor.wait_ge(s_cpA, 8 * tp + kp // 2 + 1)
                        else:
                            tensor.wait_ge(s_cpD, 8 * tp + (kp - 1) // 2 + 1)
                    c0 = k * CHUNK
                    wid = min(CHUNK, OUT_W - c0)
                    if stub:
                        nc.tensor.matmul(
                            pb[b][:rows_out, :wid],
                            mt[:3 * STUB_IN, 3 * TILE_OUT:3 * TILE_OUT + rows_out],
                            xb[t % XBUFS][:3 * STUB_IN, c0:c0 + wid],
                            start=True, stop=True,
                        ).then_inc(s_mm, 1)
                    else:
                        for dx in range(KW):
                            ins = nc.tensor.matmul(
                                pb[b][:rows_out, :wid],
                                mt[:128, dx * TILE_OUT:dx * TILE_OUT + rows_out],
                                xb[t % XBUFS][:128, c0 + dx:c0 + dx + wid],
                                start=(dx == 0),
                                stop=(dx == KW - 1),
                            )
                            if dx == KW - 1:
                                ins.then_inc(s_mm, 1)

        @block.scalar
        def _(scalar):
            scalar.dma_start(out=mt, in_=m[:]).then_inc(sm, 16)
            scalar.dma_start(
                out=xb[0][:, 0:PIECE], in_=x[0:128, 0:PIECE]
            ).then_inc(s_p0, 16)
            for t in range(N_TILES):
                rows_out = STUB_OUT if t == N_FULL else TILE_OUT
                r0 = t * TILE_OUT
                if t >= OBUFS:
                    scalar.wait_ge(sst[t - OBUFS], 32)

                def act_copy(k):
                    g = t * N_CHUNKS + k
                    c0 = k * CHUNK
                    wid = min(CHUNK, OUT_W - c0)
                    scalar.wait_ge(s_mm, g + 1)
                    nc.scalar.copy(
                        out=ob[t % OBUFS][:rows_out, c0:c0 + wid],
                        in_=pb[g % 8][:rows_out, :wid],
                    ).then_inc(s_cpA, 1)

                # store each half as soon as its 8 chunks are copied
                for k in range(0, N_CHUNKS // 2, 2):
                    act_copy(k)
                scalar.wait_ge(s_cpA, 8 * t + 4)
                scalar.wait_ge(s_cpD, 8 * t + 4)
                scalar.dma_start(
                    out=y[r0:r0 + rows_out, :HALF_W],
                    in_=ob[t % OBUFS][:rows_out, :HALF_W],
                ).then_inc(sst[t], 16)
                for k in range(N_CHUNKS // 2, N_CHUNKS, 2):
                    act_copy(k)
                scalar.wait_ge(s_cpA, 8 * (t + 1))
                scalar.wait_ge(s_cpD, 8 * (t + 1))
                scalar.dma_start(
                    out=y[r0:r0 + rows_out, HALF_W:],
                    in_=ob[t % OBUFS][:rows_out, HALF_W:OUT_W],
                ).then_inc(sst[t], 16)

        @block.vector
        def _(vector):
            for t in range(N_TILES):
                rows_out = STUB_OUT if t == N_FULL else TILE_OUT
                if t >= OBUFS:
                    vector.wait_ge(sst[t - OBUFS], 32)
                for k in range(1, N_CHUNKS, 2):
                    g = t * N_CHUNKS + k
                    c0 = k * CHUNK
                    wid = min(CHUNK, OUT_W - c0)
                    vector.wait_ge(s_mm, g + 1)
                    nc.vector.tensor_copy(
                        out=ob[t % OBUFS][:rows_out, c0:c0 + wid],
                        in_=pb[g % 8][:rows_out, :wid],
                    ).then_inc(s_cpD, 1)

    return nc


def _get_program():
    if "nc" not in _NC_CACHE:
        _NC_CACHE["nc"] = _build_program()
    return _NC_CACHE["nc"]


def _band_matrices(weight: np.ndarray) -> np.ndarray:
    """m[k, dx*126 + y] = w[k-y, dx]; stub band at cols 378..393 has
    m[18*dx + k, 378 + y] = w[k-y, dx] (K=54 packed layout)."""
    mm = np.zeros((128, MCOLS), dtype=np.float16)
    ys = np.arange(TILE_OUT)
    for dx in range(KW):
        for dy in range(KH):
            mm[ys + dy, dx * TILE_OUT + ys] = weight[dy, dx]
    ys8 = np.arange(STUB_OUT)
    for dx in range(KW):
        for dy in range(KH):
            mm[dx * STUB_IN + ys8 + dy, 3 * TILE_OUT + ys8] = weight[dy, dx]
    return mm


def _in_maps(x, weight):
    mmat = _band_matrices(weight)
    x16 = x.astype(np.float16)
    maps = []
    for i in range(N_CORES):
        r0 = i * ROWS_PER_CORE
        r1 = min(r0 + IN_ROWS_PER_CORE, H)
        shard = np.zeros((IN_ROWS_PER_CORE, W), dtype=np.float16)
        shard[: r1 - r0] = x16[r0:r1]
        maps.append({"x": shard, "m": mmat})
    return maps


def kernel(x: np.ndarray, weight: np.ndarray) -> np.ndarray:
    x = np.ascontiguousarray(np.asarray(x, dtype=np.float32))
    weight = np.asarray(weight, dtype=np.float32)
    assert x.shape == (H, W) and weight.shape == (KH, KW)

    nc = _get_program()
    res = run_bass_kernel_spmd(nc, _in_maps(x, weight),
                               core_ids=list(range(N_CORES)))

    out = np.empty((OUT_H, OUT_W), dtype=np.float32)
    for i in range(N_CORES):
        r0 = i * ROWS_PER_CORE
        keep = min(ROWS_PER_CORE, OUT_H - r0)
        out[r0:r0 + keep] = res.results[i]["y"][:keep].astype(np.float32)
    return out
